# revision 10
# baseline (speedup 1.0000x reference)
"""MoE ConditionalLayer kernel for Trainium2 (8 NeuronCores, expert-parallel).

Problem: B=4096 rows, D=1024 features, C=8 conditions (experts).  Each row is
routed to one expert's 2-layer MLP (D->D relu D->D); reference semantics also
leak relu(b1[c]) @ W2[c] + b2[c] from every *other* expert into every row
(zero-masked rows still get biases).  That leak term is row-independent given
the routed expert, so it is applied on the host as a cheap per-expert
correction; the hardware kernel computes relu(x @ W1[c] + b1[c]) @ W2[c] for
the rows of expert c.

Sharding: expert-parallel - core c owns expert c's weights and the rows routed
to it (gathered + padded to a common capacity R, transposed to feature-major).
Weights/activations are bf16 (same 1 cycle/row PE rate as full-rate fp32r,
half the HBM bytes; ~3e-3 rel err vs the 2e-2 gate).

Raw Bass (explicit engine blocks + semaphores).  Every input DMA gets its OWN
semaphore: completions on a shared queue are only approximately FIFO, so
cumulative thresholds on one semaphore race (observed on HW).

Schedule (per core), tuned against the NTFF profile:
  - ~7.4us framework preamble is fixed; a DVE memset + a few dummy matmuls
    warm the PE pstate while the first stripes are still in flight.
  - SP issues the 8 [W1 pass-0 m-tiles | xT] stripes singly (sem sa[k]) plus
    bias/W2-stripe-0; the DGE costs ~650ns fixed per dma_start, so ACT (the
    second HWDGE queue) issues the remaining W1 m-tiles and W2 stripes as 4
    grouped DMAs, gated on sa[3] so they don't steal HBM bandwidth from the
    stripe chase.
  - L1 pass 0 is k-outer (consumes stripes in arrival order).  Every later
    pass is BANK-major (full k-chain per psum bank): banks complete staggered
    ~1us apart, so DVE/ACT evacuation and the per-bank output DMAs trail the
    PE instead of serializing at pass boundaries and at the end.
  - DVE evacuates layer-1 psums with fused bias+relu into bf16 h; ACT
    evacuates layer-2 psums into bf16 staging; SP streams one output DMA per
    bank.  All cross-engine waits use per-engine in-order semaphore counts.
"""

import sys

for _p in ("/opt/trn_rl_repo", "/root/.axon_site/_ro/trn_rl_repo"):
    if _p not in sys.path:
        sys.path.append(_p)

import numpy as np

B, D, C = 4096, 1024, 8
P = 128
KT = D // P  # 8 K-tiles (and 8 M-tiles)

_NC_CACHE: dict = {}


def _build_nc(R: int, chunk: int, n_chunks: int):
    from contextlib import ExitStack

    import concourse.bass as bass
    from concourse import mybir

    f32 = mybir.dt.float32
    bf16 = mybir.dt.bfloat16
    Alu = mybir.AluOpType
    Act = mybir.ActivationFunctionType

    HALF = KT // n_chunks if n_chunks <= KT else 1  # m-tiles per pass
    NPASS = KT // HALF                              # passes per layer
    NB = HALF * n_chunks                            # psum banks per pass (<=8)
    MW = HALF * P                                   # W1 m-cols in pass-0 stripe
    assert NB <= 8
    WB = D - MW                                     # comb1b cols per stripe
    G1 = KT // 2                                    # comb1b stripes per group
    N_DUMMY = 9                                     # pstate warm-up matmuls
    TAILK = 2                                       # staggered k-steps in L1 pass 0

    # Evacuation ownership.  L1 pass 0 completes all banks at once (k-outer),
    # so its evac is split DVE(even)/ACT(odd) to halve the drain; later L1
    # passes are bank-major (staggered) and go to DVE alone.  L2 evac is ACT,
    # except the LAST pass where DVE takes the odd banks so the final banks
    # drain in parallel.  Each owner increments its own semaphore in its own
    # program order, so PE/SP waits use exact per-owner counts.
    l1_dve, l1_act = [], []          # (j, b) lists in engine program order
    for j in range(NPASS):
        for b in range(NB):
            if j == 0 and b % 2 == 1:
                l1_act.append((j, b))
            else:
                l1_dve.append((j, b))
    l2_act, l2_dve = [], []
    for j in range(NPASS):
        for b in range(NB):
            if j == NPASS - 1 and b % 2 == 1:
                l2_dve.append((j, b))
            else:
                l2_act.append((j, b))
    l1_dve_idx = {k: i + 1 for i, k in enumerate(l1_dve)}
    l1_act_idx = {k: i + 1 for i, k in enumerate(l1_act)}
    l2_act_idx = {k: i + 1 for i, k in enumerate(l2_act)}
    l2_dve_idx = {k: i + 1 for i, k in enumerate(l2_dve)}

    nc = bass.Bass()
    pkb = nc.declare_dram_parameter("pkb", [P, KT], f32, isOutput=False)
    pk2a = nc.declare_dram_parameter("pk2a", [P, D], bf16, isOutput=False)
    pk1a = nc.declare_dram_parameter("pk1a", [D, MW + R], bf16, isOutput=False)
    if NPASS > 1:
        pk1b = nc.declare_dram_parameter("pk1b", [D, WB], bf16, isOutput=False)
    pk2b = nc.declare_dram_parameter("pk2b", [D - P, D], bf16, isOutput=False)
    outT = nc.declare_dram_parameter("outT", [D, R], bf16, isOutput=True)

    with ExitStack() as ctx:
        biasb = ctx.enter_context(nc.sbuf_tensor("biasb", [P, KT], f32))
        w2s0 = ctx.enter_context(nc.sbuf_tensor("w2s0", [P, D], bf16))
        scratch = ctx.enter_context(nc.sbuf_tensor("scratch", [P, 256], bf16))
        comb1a = [ctx.enter_context(nc.sbuf_tensor(f"comb1a_{k}", [P, MW + R], bf16)) for k in range(KT)]
        # comb1b stripes live in two grouped tensors (one DMA each)
        comb1bg = [ctx.enter_context(nc.sbuf_tensor(f"comb1bg_{g}", [P, G1 * WB], bf16)) for g in range(2)] if NPASS > 1 else []
        # W2 stripes 1..7: stripes 1-3 in group 0, 4-7 in group 1
        comb2g = [ctx.enter_context(nc.sbuf_tensor("comb2g_0", [P, 3 * D], bf16)),
                  ctx.enter_context(nc.sbuf_tensor("comb2g_1", [P, 4 * D], bf16))]
        h = [ctx.enter_context(nc.sbuf_tensor(f"h_{k}", [P, R], bf16)) for k in range(KT)]
        ot = [ctx.enter_context(nc.sbuf_tensor(f"ot_{j}", [P, HALF * R], bf16)) for j in range(NPASS)]
        ps = [ctx.enter_context(nc.psum_tensor(f"ps_{b}", [P, 512], f32)) for b in range(NB)]
        wsem = ctx.enter_context(nc.semaphore("wsem"))
        s0 = ctx.enter_context(nc.semaphore("s0"))
        sa = [ctx.enter_context(nc.semaphore(f"sa{k}")) for k in range(KT)]
        sb = [ctx.enter_context(nc.semaphore(f"sb{g}")) for g in range(2)] if NPASS > 1 else []
        sc = [ctx.enter_context(nc.semaphore(f"sc{g}")) for g in range(2)]
        psem = ctx.enter_context(nc.semaphore("psem"))
        vsem = ctx.enter_context(nc.semaphore("vsem"))
        v2sem = ctx.enter_context(nc.semaphore("v2sem"))
        asem = ctx.enter_context(nc.semaphore("asem"))
        a2sem = ctx.enter_context(nc.semaphore("a2sem"))
        osem = ctx.enter_context(nc.semaphore("osem"))
        block = ctx.enter_context(nc.Block())

        def wait_l1_evac(tensor, pairs):
            """Wait until every (j, b) L1-psum in `pairs` has been evacuated."""
            need_v = [l1_dve_idx[p] for p in pairs if p in l1_dve_idx]
            need_v2 = [l1_act_idx[p] for p in pairs if p in l1_act_idx]
            if need_v:
                tensor.wait_ge(vsem, max(need_v))
            if need_v2:
                tensor.wait_ge(v2sem, max(need_v2))

        def comb1b_ap(k, col0, col1):
            g, kk = divmod(k, G1)  # k is the 0..KT-1 stripe index
            return comb1bg[g][:, kk * WB + col0:kk * WB + col1]

        def comb2_ap(k, col0, col1):
            # k = 1..KT-1 -> group 0 holds 1..3, group 1 holds 4..7
            if k <= 3:
                return comb2g[0][:, (k - 1) * D + col0:(k - 1) * D + col1]
            return comb2g[1][:, (k - 4) * D + col0:(k - 4) * D + col1]

        # banks enumerate (c, i); bank b = c*HALF + i; m-tile = j*HALF + i
        def banks():
            return [(c * HALF + i, c, i) for c in range(n_chunks) for i in range(HALF)]

        def l1_w(j, k, m):
            if j == 0:
                return comb1a[k][:, m * P:(m + 1) * P]
            return comb1b_ap(k, (m - HALF) * P, (m - HALF + 1) * P)

        def l2_w(k, m):
            if k == 0:
                return w2s0[:, m * P:(m + 1) * P]
            return comb2_ap(k, m * P, (m + 1) * P)

        @block.sync
        def _(sync):
            for k in range(KT):
                sync.dma_start(
                    out=comb1a[k][:], in_=pk1a[k * P:(k + 1) * P, :]
                ).then_inc(sa[k], 16)
            sync.dma_start(out=biasb[:], in_=pkb[:]).then_inc(s0, 16)
            sync.dma_start(out=w2s0[:], in_=pk2a[:]).then_inc(s0, 16)
            for j in range(NPASS):
                for b, c, i in banks():
                    if (j, b) in l2_act_idx:
                        sync.wait_ge(asem, l2_act_idx[(j, b)])
                    else:
                        sync.wait_ge(a2sem, l2_dve_idx[(j, b)])
                    m = j * HALF + i
                    sync.dma_start(
                        out=outT[m * P:(m + 1) * P, c * chunk:(c + 1) * chunk],
                        in_=ot[j][:, i * R + c * chunk:i * R + (c + 1) * chunk],
                    ).then_inc(osem, 16)
            sync.wait_ge(osem, NPASS * NB * 16)

        @block.tensor
        def _(tensor):
            # pstate warm-up on memset scratch while first stripes fly
            tensor.wait_ge(wsem, 1)
            for _ in range(N_DUMMY):
                tensor.matmul(ps[0][:, :256], scratch[:, :P], scratch[:, :256],
                              start=True, stop=True)
            # layer 1 pass 0: k-outer chasing stripes, but the last TAILK
            # k-steps go bank-major so the psum stop flags stagger and the
            # DVE/ACT evacuation drains behind the PE instead of after it.
            for k in range(KT - TAILK):
                tensor.wait_ge(sa[k], 16)
                for b, c, i in banks():
                    tensor.matmul(
                        ps[b][:, :chunk], l1_w(0, k, i),
                        comb1a[k][:, MW + c * chunk:MW + (c + 1) * chunk],
                        start=(k == 0), stop=False,
                    )
            for k in range(KT - TAILK, KT):
                tensor.wait_ge(sa[k], 16)
            for b, c, i in banks():
                for k in range(KT - TAILK, KT):
                    mm = tensor.matmul(
                        ps[b][:, :chunk], l1_w(0, k, i),
                        comb1a[k][:, MW + c * chunk:MW + (c + 1) * chunk],
                        start=False, stop=(k == KT - 1),
                    )
                    if k == KT - 1:
                        mm.then_inc(psem, 1)
            # layer 1 passes 1..: bank-major
            for j in range(1, NPASS):
                if j == 1:
                    tensor.wait_ge(sb[0], 16)
                    tensor.wait_ge(sb[1], 16)
                for b, c, i in banks():
                    wait_l1_evac(tensor, [(j - 1, b)])
                    m = j * HALF + i
                    for k in range(KT):
                        mm = tensor.matmul(
                            ps[b][:, :chunk], l1_w(j, k, m),
                            comb1a[k][:, MW + c * chunk:MW + (c + 1) * chunk],
                            start=(k == 0), stop=(k == KT - 1),
                        )
                        if k == KT - 1:
                            mm.then_inc(psem, 1)
            # layer 2: bank-major
            for j in range(NPASS):
                if j == 0:
                    tensor.wait_ge(s0, 32)
                    tensor.wait_ge(sc[0], 16)
                    tensor.wait_ge(sc[1], 16)
                for b, c, i in banks():
                    if j == 0:
                        # h chunk c fully written (covers psum bank b free too)
                        wait_l1_evac(tensor, [(jj, c * HALF + ii)
                                              for jj in range(NPASS)
                                              for ii in range(HALF)])
                    else:
                        tensor.wait_ge(asem, l2_act_idx[(j - 1, b)])
                    m = j * HALF + i
                    for k in range(KT):
                        mm = tensor.matmul(
                            ps[b][:, :chunk], l2_w(k, m),
                            h[k][:, c * chunk:(c + 1) * chunk],
                            start=(k == 0), stop=(k == KT - 1),
                        )
                        if k == KT - 1:
                            mm.then_inc(psem, 1)

        @block.vector
        def _(vector):
            vector.memset(scratch[:], 0.0).then_inc(wsem, 1)
            vector.wait_ge(s0, 32)  # bias tile landed
            for j, b in l1_dve:
                c, i = divmod(b, HALF)
                m = j * HALF + i
                vector.wait_ge(psem, j * NB + b + 1)
                vector.tensor_scalar(
                    h[m][:, c * chunk:(c + 1) * chunk], ps[b][:, :chunk],
                    biasb[:, m:m + 1], 0.0, Alu.add, Alu.max,
                ).then_inc(vsem, 1)
            for j, b in l2_dve:
                c, i = divmod(b, HALF)
                vector.wait_ge(psem, (NPASS + j) * NB + b + 1)
                vector.tensor_scalar_add(
                    ot[j][:, i * R + c * chunk:i * R + (c + 1) * chunk],
                    ps[b][:, :chunk], 0.0,
                ).then_inc(a2sem, 1)

        @block.scalar
        def _(scalar):
            # second HWDGE queue: W1 remaining m-tiles + W2 stripes, grouped.
            # Gate on sa[3] so these big transfers trail the stripe chase.
            scalar.wait_ge(sa[3], 16)
            if NPASS > 1:
                for g in range(2):
                    scalar.dma_start(
                        out=comb1bg[g][:].rearrange("p (g w) -> p g w", g=G1),
                        in_=pk1b[g * G1 * P:(g + 1) * G1 * P, :]
                            .rearrange("(g p) w -> p g w", p=P),
                    ).then_inc(sb[g], 16)
            scalar.dma_start(
                out=comb2g[0][:].rearrange("p (g w) -> p g w", g=3),
                in_=pk2b[0:3 * P, :].rearrange("(g p) w -> p g w", p=P),
            ).then_inc(sc[0], 16)
            scalar.dma_start(
                out=comb2g[1][:].rearrange("p (g w) -> p g w", g=4),
                in_=pk2b[3 * P:7 * P, :].rearrange("(g p) w -> p g w", p=P),
            ).then_inc(sc[1], 16)
            if l1_act:
                scalar.wait_ge(s0, 32)  # bias tile landed
            for j, b in l1_act:
                c, i = divmod(b, HALF)
                m = j * HALF + i
                scalar.wait_ge(psem, j * NB + b + 1)
                scalar.activation(
                    h[m][:, c * chunk:(c + 1) * chunk], ps[b][:, :chunk],
                    Act.Relu, bias=biasb[:, m:m + 1],
                ).then_inc(v2sem, 1)
            for j, b in l2_act:
                c, i = divmod(b, HALF)
                scalar.wait_ge(psem, (NPASS + j) * NB + b + 1)
                scalar.activation(
                    ot[j][:, i * R + c * chunk:i * R + (c + 1) * chunk],
                    ps[b][:, :chunk], Act.Copy,
                ).then_inc(asem, 1)

    return nc


def _capacity(maxc: int):
    n_chunks = max(1, -(-maxc // 512))
    chunk = -(-maxc // n_chunks)
    chunk = -(-chunk // 16) * 16
    return chunk * n_chunks, chunk, n_chunks


def kernel(x, cond_ids, W1, b1, W2, b2, _want_trace=False):
    import ml_dtypes
    from concourse.bass_utils import run_bass_kernel_spmd

    bf = ml_dtypes.bfloat16
    x = np.ascontiguousarray(np.asarray(x, dtype=np.float32))
    cond_ids = np.asarray(cond_ids)
    cid = cond_ids.astype(np.int64)
    W1 = np.asarray(W1, dtype=np.float32)
    b1 = np.asarray(b1, dtype=np.float32)
    W2 = np.asarray(W2, dtype=np.float32)
    b2 = np.asarray(b2, dtype=np.float32)

    counts = np.bincount(cid, minlength=C)
    R, chunk, n_chunks = _capacity(int(counts.max()))
    HALF = KT // n_chunks if n_chunks <= KT else 1
    NPASS = KT // HALF
    MW = HALF * P

    key = (R, chunk, n_chunks)
    if key not in _NC_CACHE:
        _NC_CACHE[key] = _build_nc(R, chunk, n_chunks)
    nc = _NC_CACHE[key]

    order = np.argsort(cid, kind="stable")
    bounds = np.concatenate([[0], np.cumsum(counts)])

    W1b = W1.astype(bf)
    W2b = W2.astype(bf)
    pkb = np.ascontiguousarray(b1.reshape(C, KT, P).transpose(0, 2, 1))
    pk2a = np.ascontiguousarray(W2b[:, :P, :])
    pk2b = np.ascontiguousarray(W2b[:, P:, :])
    pk1a = np.zeros((C, D, MW + R), bf)
    pk1a[:, :, :MW] = W1b[:, :, :MW]
    if NPASS > 1:
        pk1b = np.ascontiguousarray(W1b[:, :, MW:])
    for c in range(C):
        rows = order[bounds[c]:bounds[c + 1]]
        if len(rows):
            pk1a[c, :, MW:MW + len(rows)] = x[rows].T.astype(bf)

    in_maps = []
    for c in range(C):
        m = {"pkb": pkb[c], "pk2a": pk2a[c], "pk1a": pk1a[c], "pk2b": pk2b[c]}
        if NPASS > 1:
            m["pk1b"] = pk1b[c]
        in_maps.append(m)
    res = run_bass_kernel_spmd(nc, in_maps, list(range(C)), trace=_want_trace)

    out = np.empty((B, D), np.float32)
    for c in range(C):
        rows = order[bounds[c]:bounds[c + 1]]
        if len(rows):
            out[rows] = res.results[c]["outT"][:, :len(rows)].T.astype(np.float32)

    # Reference leaks every expert's bias response through zero-masked rows:
    # out_true[b] = relu(x@W1[cb]+b1[cb])@W2[cb] + b2[cb] + sum_{c!=cb} z[c],
    # z[c] = relu(b1[c]) @ W2[c] + b2[c].  Kernel computed the first term
    # minus b2; add the rest here (exactly zero for zero biases).
    if b1.any() or b2.any():
        z = np.einsum("cd,cde->ce", np.maximum(b1, 0.0), W2) + b2
        corr = b2 + z.sum(axis=0)[None, :] - z
        out += corr[cid]

    if _want_trace:
        kernel._last_results = res
    return out


# revision 18
# speedup vs baseline: 1.1560x; 1.1560x over previous
"""MoE ConditionalLayer kernel for Trainium2 (8 NeuronCores, expert-parallel).

Problem: B=4096 rows, D=1024 features, C=8 conditions (experts).  Each row is
routed to one expert's 2-layer MLP (D->D relu D->D); reference semantics also
leak relu(b1[c]) @ W2[c] + b2[c] from every *other* expert into every row
(zero-masked rows still get biases).  That leak term is row-independent given
the routed expert, so it is applied on the host as a cheap per-expert
correction; the hardware kernel computes relu(x @ W1[c] + b1[c]) @ W2[c] for
the rows of expert c.

Sharding: expert-parallel - core c owns expert c's weights and the rows routed
to it (gathered + padded to a common capacity R, transposed to feature-major).
Weights/activations are bf16 (same 1 cycle/row PE rate as full-rate fp32r,
half the HBM bytes; ~3e-3 rel err vs the 2e-2 gate).

Raw Bass (explicit engine blocks + semaphores).  Every input DMA gets its OWN
semaphore: completions on a shared queue are only approximately FIFO, so
cumulative thresholds on one semaphore race (observed on HW).

Schedule (per core), tuned against the NTFF profile:
  - ~7.4us framework preamble is fixed; a DVE memset + a few dummy matmuls
    warm the PE pstate while the first stripes are still in flight.
  - SP issues the 8 [W1 pass-0 m-tiles | xT] stripes singly (sem sa[k]) plus
    bias/W2-stripe-0; the DGE costs ~650ns fixed per dma_start, so ACT (the
    second HWDGE queue) issues the remaining W1 m-tiles and W2 stripes as 4
    grouped DMAs, gated on sa[3] so they don't steal HBM bandwidth from the
    stripe chase.
  - L1 pass 0 is k-outer (consumes stripes in arrival order).  Every later
    pass is BANK-major (full k-chain per psum bank): banks complete staggered
    ~1us apart, so DVE/ACT evacuation and the per-bank output DMAs trail the
    PE instead of serializing at pass boundaries and at the end.
  - DVE evacuates layer-1 psums with fused bias+relu into bf16 h; ACT
    evacuates layer-2 psums into bf16 staging; SP streams one output DMA per
    bank.  All cross-engine waits use per-engine in-order semaphore counts.
"""

import sys

for _p in ("/opt/trn_rl_repo", "/root/.axon_site/_ro/trn_rl_repo"):
    if _p not in sys.path:
        sys.path.append(_p)

import numpy as np

B, D, C = 4096, 1024, 8
P = 128
KT = D // P  # 8 K-tiles (and 8 M-tiles)

_NC_CACHE: dict = {}


def _build_nc(R: int, chunk: int, n_chunks: int):
    from contextlib import ExitStack

    import concourse.bass as bass
    from concourse import mybir

    f32 = mybir.dt.float32
    bf16 = mybir.dt.bfloat16
    Alu = mybir.AluOpType
    Act = mybir.ActivationFunctionType

    HALF = KT // n_chunks if n_chunks <= KT else 1  # m-tiles per pass
    NPASS = KT // HALF                              # passes per layer
    NB = HALF * n_chunks                            # psum banks per pass (<=8)
    MW = HALF * P                                   # W1 m-cols in pass-0 stripe
    assert NB <= 8
    WB = D - MW                                     # comb1b cols per stripe
    G1 = KT // 2                                    # comb1b stripes per group
    N_DUMMY = 5                                     # pstate warm-up matmuls

    nc = bass.Bass()
    pkb = nc.declare_dram_parameter("pkb", [P, KT], f32, isOutput=False)
    pk2a = nc.declare_dram_parameter("pk2a", [P, D], bf16, isOutput=False)
    pk1a = nc.declare_dram_parameter("pk1a", [D, MW + R], bf16, isOutput=False)
    if NPASS > 1:
        pk1b = nc.declare_dram_parameter("pk1b", [D, WB], bf16, isOutput=False)
    pk2b = nc.declare_dram_parameter("pk2b", [D - P, D], bf16, isOutput=False)
    outT = nc.declare_dram_parameter("outT", [D, R], bf16, isOutput=True)

    with ExitStack() as ctx:
        biasb = ctx.enter_context(nc.sbuf_tensor("biasb", [P, KT], f32))
        w2s0 = ctx.enter_context(nc.sbuf_tensor("w2s0", [P, D], bf16))
        scratch = ctx.enter_context(nc.sbuf_tensor("scratch", [P, 512], bf16))
        comb1a = [ctx.enter_context(nc.sbuf_tensor(f"comb1a_{k}", [P, MW + R], bf16)) for k in range(KT)]
        # comb1b stripes live in two grouped tensors (one DMA each)
        comb1bg = [ctx.enter_context(nc.sbuf_tensor(f"comb1bg_{g}", [P, G1 * WB], bf16)) for g in range(2)] if NPASS > 1 else []
        # W2 stripes 1..7: stripes 1-3 in group 0, 4-7 in group 1
        comb2g = [ctx.enter_context(nc.sbuf_tensor("comb2g_0", [P, 3 * D], bf16)),
                  ctx.enter_context(nc.sbuf_tensor("comb2g_1", [P, 4 * D], bf16))]
        h = [ctx.enter_context(nc.sbuf_tensor(f"h_{k}", [P, R], bf16)) for k in range(KT)]
        ot = [ctx.enter_context(nc.sbuf_tensor(f"ot_{j}", [P, HALF * R], bf16)) for j in range(NPASS)]
        ps = [ctx.enter_context(nc.psum_tensor(f"ps_{b}", [P, 512], f32)) for b in range(NB)]
        wsem = ctx.enter_context(nc.semaphore("wsem"))
        s0 = ctx.enter_context(nc.semaphore("s0"))
        sa = [ctx.enter_context(nc.semaphore(f"sa{k}")) for k in range(KT)]
        sb = [ctx.enter_context(nc.semaphore(f"sb{g}")) for g in range(2)] if NPASS > 1 else []
        sc = [ctx.enter_context(nc.semaphore(f"sc{g}")) for g in range(2)]
        psem = ctx.enter_context(nc.semaphore("psem"))
        vsem = ctx.enter_context(nc.semaphore("vsem"))
        asem = ctx.enter_context(nc.semaphore("asem"))
        osem = ctx.enter_context(nc.semaphore("osem"))
        block = ctx.enter_context(nc.Block())

        def comb1b_ap(k, col0, col1):
            g, kk = divmod(k, G1)  # k is the 0..KT-1 stripe index
            return comb1bg[g][:, kk * WB + col0:kk * WB + col1]

        def comb2_ap(k, col0, col1):
            # k = 1..KT-1 -> group 0 holds 1..3, group 1 holds 4..7
            if k <= 3:
                return comb2g[0][:, (k - 1) * D + col0:(k - 1) * D + col1]
            return comb2g[1][:, (k - 4) * D + col0:(k - 4) * D + col1]

        # banks enumerate (c, i); bank b = c*HALF + i; m-tile = j*HALF + i
        def banks():
            return [(c * HALF + i, c, i) for c in range(n_chunks) for i in range(HALF)]

        def l1_w(j, k, m):
            if j == 0:
                return comb1a[k][:, m * P:(m + 1) * P]
            return comb1b_ap(k, (m - HALF) * P, (m - HALF + 1) * P)

        def l2_w(k, m):
            if k == 0:
                return w2s0[:, m * P:(m + 1) * P]
            return comb2_ap(k, m * P, (m + 1) * P)

        @block.sync
        def _(sync):
            for k in range(KT):
                sync.dma_start(
                    out=comb1a[k][:], in_=pk1a[k * P:(k + 1) * P, :]
                ).then_inc(sa[k], 16)
            sync.dma_start(out=biasb[:], in_=pkb[:]).then_inc(s0, 16)
            sync.dma_start(out=w2s0[:], in_=pk2a[:]).then_inc(s0, 16)
            for j in range(NPASS):
                for b, c, i in banks():
                    sync.wait_ge(asem, j * NB + b + 1)
                    m = j * HALF + i
                    sync.dma_start(
                        out=outT[m * P:(m + 1) * P, c * chunk:(c + 1) * chunk],
                        in_=ot[j][:, i * R + c * chunk:i * R + (c + 1) * chunk],
                    ).then_inc(osem, 16)
            sync.wait_ge(osem, NPASS * NB * 16)

        @block.tensor
        def _(tensor):
            # pstate warm-up on memset scratch while first stripes fly
            tensor.wait_ge(wsem, 1)
            for _ in range(N_DUMMY):
                tensor.matmul(ps[0][:, :512], scratch[:, :P], scratch[:, :512],
                              start=True, stop=True)
            # layer 1 pass 0: k-outer, chasing stripes
            for k in range(KT):
                tensor.wait_ge(sa[k], 16)
                for b, c, i in banks():
                    mm = tensor.matmul(
                        ps[b][:, :chunk], l1_w(0, k, i),
                        comb1a[k][:, MW + c * chunk:MW + (c + 1) * chunk],
                        start=(k == 0), stop=(k == KT - 1),
                    )
                    if k == KT - 1:
                        mm.then_inc(psem, 1)
            # layer 1 passes 1..: bank-major
            for j in range(1, NPASS):
                if j == 1:
                    tensor.wait_ge(sb[0], 16)
                    tensor.wait_ge(sb[1], 16)
                for b, c, i in banks():
                    tensor.wait_ge(vsem, (j - 1) * NB + b + 1)
                    m = j * HALF + i
                    for k in range(KT):
                        mm = tensor.matmul(
                            ps[b][:, :chunk], l1_w(j, k, m),
                            comb1a[k][:, MW + c * chunk:MW + (c + 1) * chunk],
                            start=(k == 0), stop=(k == KT - 1),
                        )
                        if k == KT - 1:
                            mm.then_inc(psem, 1)
            # layer 2: bank-major
            for j in range(NPASS):
                if j == 0:
                    tensor.wait_ge(s0, 32)
                    tensor.wait_ge(sc[0], 16)
                    tensor.wait_ge(sc[1], 16)
                for b, c, i in banks():
                    if j == 0:
                        # h chunk c fully written (covers psum bank b free too)
                        tensor.wait_ge(vsem, (NPASS - 1) * NB + c * HALF + HALF)
                    else:
                        tensor.wait_ge(asem, (j - 1) * NB + b + 1)
                    m = j * HALF + i
                    for k in range(KT):
                        mm = tensor.matmul(
                            ps[b][:, :chunk], l2_w(k, m),
                            h[k][:, c * chunk:(c + 1) * chunk],
                            start=(k == 0), stop=(k == KT - 1),
                        )
                        if k == KT - 1:
                            mm.then_inc(psem, 1)

        @block.vector
        def _(vector):
            vector.memset(scratch[:], 0.0).then_inc(wsem, 1)
            vector.wait_ge(s0, 32)  # bias tile landed
            for j in range(NPASS):
                for b, c, i in banks():
                    m = j * HALF + i
                    vector.wait_ge(psem, j * NB + b + 1)
                    vector.tensor_scalar(
                        h[m][:, c * chunk:(c + 1) * chunk], ps[b][:, :chunk],
                        biasb[:, m:m + 1], 0.0, Alu.add, Alu.max,
                    ).then_inc(vsem, 1)

        @block.scalar
        def _(scalar):
            # second HWDGE queue: W1 remaining m-tiles + W2 stripes, grouped.
            # Gate on sa[3] so these big transfers trail the stripe chase.
            scalar.wait_ge(sa[3], 16)
            if NPASS > 1:
                for g in range(2):
                    scalar.dma_start(
                        out=comb1bg[g][:].rearrange("p (g w) -> p g w", g=G1),
                        in_=pk1b[g * G1 * P:(g + 1) * G1 * P, :]
                            .rearrange("(g p) w -> p g w", p=P),
                    ).then_inc(sb[g], 16)
            scalar.dma_start(
                out=comb2g[0][:].rearrange("p (g w) -> p g w", g=3),
                in_=pk2b[0:3 * P, :].rearrange("(g p) w -> p g w", p=P),
            ).then_inc(sc[0], 16)
            scalar.dma_start(
                out=comb2g[1][:].rearrange("p (g w) -> p g w", g=4),
                in_=pk2b[3 * P:7 * P, :].rearrange("(g p) w -> p g w", p=P),
            ).then_inc(sc[1], 16)
            for j in range(NPASS):
                for b, c, i in banks():
                    scalar.wait_ge(psem, (NPASS + j) * NB + b + 1)
                    scalar.activation(
                        ot[j][:, i * R + c * chunk:i * R + (c + 1) * chunk],
                        ps[b][:, :chunk], Act.Copy,
                    ).then_inc(asem, 1)

    return nc


def _capacity(maxc: int):
    n_chunks = max(1, -(-maxc // 512))
    chunk = -(-maxc // n_chunks)
    chunk = -(-chunk // 16) * 16
    return chunk * n_chunks, chunk, n_chunks


def kernel(x, cond_ids, W1, b1, W2, b2, _want_trace=False):
    import ml_dtypes
    from concourse.bass_utils import run_bass_kernel_spmd

    bf = ml_dtypes.bfloat16
    x = np.ascontiguousarray(np.asarray(x, dtype=np.float32))
    cond_ids = np.asarray(cond_ids)
    cid = cond_ids.astype(np.int64)
    W1 = np.asarray(W1, dtype=np.float32)
    b1 = np.asarray(b1, dtype=np.float32)
    W2 = np.asarray(W2, dtype=np.float32)
    b2 = np.asarray(b2, dtype=np.float32)

    counts = np.bincount(cid, minlength=C)
    R, chunk, n_chunks = _capacity(int(counts.max()))
    HALF = KT // n_chunks if n_chunks <= KT else 1
    NPASS = KT // HALF
    MW = HALF * P

    key = (R, chunk, n_chunks)
    if key not in _NC_CACHE:
        _NC_CACHE[key] = _build_nc(R, chunk, n_chunks)
    nc = _NC_CACHE[key]

    order = np.argsort(cid, kind="stable")
    bounds = np.concatenate([[0], np.cumsum(counts)])

    W1b = W1.astype(bf)
    W2b = W2.astype(bf)
    pkb = np.ascontiguousarray(b1.reshape(C, KT, P).transpose(0, 2, 1))
    pk2a = np.ascontiguousarray(W2b[:, :P, :])
    pk2b = np.ascontiguousarray(W2b[:, P:, :])
    pk1a = np.zeros((C, D, MW + R), bf)
    pk1a[:, :, :MW] = W1b[:, :, :MW]
    if NPASS > 1:
        pk1b = np.ascontiguousarray(W1b[:, :, MW:])
    for c in range(C):
        rows = order[bounds[c]:bounds[c + 1]]
        if len(rows):
            pk1a[c, :, MW:MW + len(rows)] = x[rows].T.astype(bf)

    in_maps = []
    for c in range(C):
        m = {"pkb": pkb[c], "pk2a": pk2a[c], "pk1a": pk1a[c], "pk2b": pk2b[c]}
        if NPASS > 1:
            m["pk1b"] = pk1b[c]
        in_maps.append(m)
    res = run_bass_kernel_spmd(nc, in_maps, list(range(C)), trace=_want_trace)

    out = np.empty((B, D), np.float32)
    for c in range(C):
        rows = order[bounds[c]:bounds[c + 1]]
        if len(rows):
            out[rows] = res.results[c]["outT"][:, :len(rows)].T.astype(np.float32)

    # Reference leaks every expert's bias response through zero-masked rows:
    # out_true[b] = relu(x@W1[cb]+b1[cb])@W2[cb] + b2[cb] + sum_{c!=cb} z[c],
    # z[c] = relu(b1[c]) @ W2[c] + b2[c].  Kernel computed the first term
    # minus b2; add the rest here (exactly zero for zero biases).
    if b1.any() or b2.any():
        z = np.einsum("cd,cde->ce", np.maximum(b1, 0.0), W2) + b2
        corr = b2 + z.sum(axis=0)[None, :] - z
        out += corr[cid]

    if _want_trace:
        kernel._last_results = res
    return out
